# revision 27
# baseline (speedup 1.0000x reference)
"""Causal self-attention (B=2, T=2048, C=1024, H=16) on 8 TRN2 NeuronCores.

Megatron-style tensor parallelism over heads: each core computes 2 of the 16
heads (Wq/Wk/Wv column-sharded, Wo row-sharded) and produces a partial output
projection; the partials are summed on the host (the all-reduce).

Per-core device dataflow (everything kept transposed so the PE contraction dim
is always the partition dim, no on-device transposes of x needed):
  xTt [tb*128+p, ct*512+t] host-pretiled x so each DMA descriptor moves a
      contiguous 8KB per partition; prefetched two t-blocks ahead
  QT/KT/VT = Wqkv_locT.T @ xT  (bf16 matmuls, K-tiled over C, one merged
      weight tile; PSUM drained by DVE tensor_scalar_add which also casts)
  V tiles  = PE-transpose of VT; per-head weight blocks padded to 128 cols:
      [V_h(64) | ones | junk(63)] so FWL stays enabled and the ones column
      accumulates the softmax row-sums
  S^T      = K_loc @ Q_loc^T per (batch, 128-j-tile, 512-i-block); the two
      heads run as concurrent PE row groups (contraction D=64 at rows 0-63 /
      64-127) into separate PSUM banks; on diagonal j-tiles the moving
      i-range is trimmed to the causal suffix
  P^T      = exp(S^T / 8) on ACT, one instruction per j-tile (no max
      subtraction needed: |S| < ~4); causal tri-mask multiply only on the
      128-wide diagonal chunk of each head
  O^T|s    = [V-block].T @ P^T accumulated over j in PSUM; O_h at partitions
      0-63, s_h at partition 64
  ylocT    = O^T * (1/s): s rows copied to partition 0 (ACT+DVE in
      parallel), DVE reciprocal, gpsimd partition-broadcast per head, then
      two DVE mults (head1 writes partitions 64-127 via output base shift)
  yT_part  = Wo_locT.T @ ylocT -> bf16 casts alternate ACT/DVE, batched
      DMA per 4 co-tiles
Host: y = (sum_cores yT_part).T + bo, reshape to [B, T, C].

Scheduling: emission interleaves qkv(tb) -> outproj(block tb-1) ->
attention(block tb); the attention chain is wrapped in tc.high_priority()
so the ACT exp stream stays dense while qkv/outproj matmuls fill PE gaps,
and the block-boundary normalization chain overlaps the next block's
S/exp pipeline and the previous block's output projection.
"""
import sys

if "/opt/trn_rl_repo" not in sys.path:
    sys.path.insert(0, "/opt/trn_rl_repo")

import numpy as np

import concourse.bass as bass
import concourse.tile as tile
from concourse import bacc
from concourse import mybir
from concourse.bass_utils import run_bass_kernel_spmd

F32 = mybir.dt.float32
BF16 = mybir.dt.bfloat16
AF = mybir.ActivationFunctionType
ALU = mybir.AluOpType

B, T, C, H = 2, 2048, 1024, 16
D = C // H          # 64
NCORES = 8
HL = H // NCORES    # 2 local heads
CL = C // NCORES    # 128 local channels
BT = B * T          # 4096
TB = 512            # t-block (matmul moving width, fp32 psum max)
NTB = BT // TB      # 8
NKT = C // 128      # 8 contraction tiles for projections
IB = T // TB        # 4 i-blocks per batch
NJT = T // 128      # 16 j-tiles per batch
VW = 256            # V cols per 128-j group: 2 heads x 128 padded weight cols


def build_nc() -> bass.Bass:
    nc = bacc.Bacc()

    xt_d = nc.declare_dram_parameter("xTt", [NTB * 128, NKT * TB], BF16,
                                     isOutput=False)
    wqkv_d = nc.declare_dram_parameter("wqkv", [128, 3 * C], BF16,
                                       isOutput=False)
    woT_d = nc.declare_dram_parameter("woT", [CL, C], BF16, isOutput=False)
    bqkv_d = nc.declare_dram_parameter("bqkv", [128, 3], F32, isOutput=False)
    it_d = nc.declare_dram_parameter("identri", [128, 256], BF16,
                                     isOutput=False)
    yT_d = nc.declare_dram_parameter("yTt", [NTB * 8 * 128, TB], BF16,
                                     isOutput=True)

    with tile.TileContext(nc) as tc:
        with (
            tc.tile_pool(name="const", bufs=1) as const,
            tc.tile_pool(name="work", bufs=2) as work,
            tc.tile_pool(name="psum", bufs=2, space="PSUM") as psum,
        ):
            # ---------------- constants / persistent state ----------------
            wqkv_sb = const.tile([128, 3 * C], BF16)
            bqkv_sb = const.tile([128, 3], F32)

            # first x block, split so the very first matmul starts early
            xts = []
            for tb in range(NTB):
                xts.append(work.tile([128, NKT * TB], BF16, tag="xt", bufs=3,
                                     name=f"xt_{tb}"))

            def emit_x_dma(tb):
                rows = slice(tb * 128, (tb + 1) * 128)
                if tb == 0:
                    nc.sync.dma_start(xts[0][:, 0:TB], xt_d[rows, 0:TB])
                    nc.sync.dma_start(xts[0][:, TB:4 * TB], xt_d[rows, TB:4 * TB])
                    nc.sync.dma_start(xts[0][:, 4 * TB:], xt_d[rows, 4 * TB:])
                else:
                    nc.sync.dma_start(xts[tb][:, :], xt_d[rows, :])

            emit_x_dma(0)
            nc.sync.dma_start(wqkv_sb[:, 0:128], wqkv_d[:, 0:128])
            nc.sync.dma_start(wqkv_sb[:, 128:C], wqkv_d[:, 128:C])
            nc.sync.dma_start(bqkv_sb[:, :], bqkv_d[:, :])
            nc.sync.dma_start(wqkv_sb[:, C:3 * C], wqkv_d[:, C:3 * C])
            it_sb = const.tile([128, 256], BF16)
            nc.sync.dma_start(it_sb[:, :], it_d[:, :])
            id_sb = it_sb[:, 0:128]
            tri_sb = it_sb[:, 128:256]
            wo_sb = const.tile([128, C], BF16)
            nc.sync.dma_start(wo_sb[:, :], woT_d[:, :])

            QT = const.tile([128, BT], BF16)
            KT = const.tile([128, BT], BF16)
            ylocT = const.tile([128, BT], BF16)
            V = const.tile([128, (BT // 128) * VW], BF16)
            # zero-fill once (junk weight cols stay inert), then set the
            # per-head ones columns (bf16 1.0 = 0x3F80)
            nc.gpsimd.memset(V[:, :].bitcast(mybir.dt.uint16), 0)
            for _jg in range(BT // 128):
                for _c in (_jg * VW + 64, _jg * VW + 192):
                    nc.gpsimd.memset(V[:, _c:_c + 1].bitcast(mybir.dt.uint16),
                                     0x3F80)

            # ---------------- phase 1: Q/K/V projections -------------------
            def emit_qkv(tb, prefetch=()):
                tcols = slice(tb * TB, (tb + 1) * TB)
                xt = xts[tb]
                for ptb in prefetch:
                    emit_x_dma(ptb)
                for wi, which in enumerate(("q", "k", "v")):
                    b_sb = bqkv_sb[:, wi:wi + 1]
                    ps = psum.tile([128, TB], F32, tag="mm", name=f"ps_{which}_{tb}")
                    for ct in range(NKT):
                        nc.tensor.matmul(
                            ps[:, :],
                            wqkv_sb[:, wi * C + ct * 128:wi * C + (ct + 1) * 128],
                            xt[:, ct * TB:(ct + 1) * TB],
                            start=(ct == 0), stop=(ct == NKT - 1),
                        )
                    if which == "q":
                        nc.vector.tensor_scalar_add(QT[:, tcols], ps[:, :],
                                                    b_sb[:, :])
                    elif which == "k":
                        nc.vector.tensor_scalar_add(KT[:, tcols], ps[:, :],
                                                    b_sb[:, :])
                    else:
                        vt_sb = work.tile([128, TB], BF16, tag="vtsb",
                                          name=f"vt_{tb}")
                        nc.vector.tensor_scalar_add(vt_sb[:, :], ps[:, :],
                                                    b_sb[:, :])
                        for q in range(4):
                            jg = tb * 4 + q
                            tp = psum.tile([128, 128], BF16, tag="mm",
                                           name=f"tp_{jg}")
                            off = jg * VW
                            nc.tensor.transpose(
                                tp[:, :],
                                vt_sb[:, q * 128:(q + 1) * 128],
                                id_sb,
                            )
                            # per-head blocks: [V(64) | ones | junk(63)]
                            nc.vector.tensor_copy(V[:, off:off + 64],
                                                  tp[:, 0:64])
                            nc.vector.tensor_copy(V[:, off + 128:off + 192],
                                                  tp[:, 64:128])

            # ------- phase 2+3: attention + output projection per i-block ----
            def emit_attention(b, ib):
                # The jt loop is ACT(exp)-paced; keep its whole chain ahead
                # of qkv/outproj fill work in the scheduler's priority heap.
                with tc.high_priority():
                    _emit_attention(b, ib)

            def _emit_attention(b, ib):
                blk = b * IB + ib
                i0 = b * T + ib * TB
                icols = slice(i0, i0 + TB)
                njt = 4 * (ib + 1)
                ot0 = psum.tile([128, TB], F32, tag="ot", bufs=2,
                                name=f"ot0_{b}_{ib}")
                ot1 = psum.tile([128, TB], F32, tag="ot", bufs=2,
                                name=f"ot1_{b}_{ib}")
                for jt in range(njt):
                    jg = b * NJT + jt
                    q = jt - (njt - 4)      # >= 0 -> diagonal 512-block
                    w = TB if q < 0 else TB - 128 * q
                    ioff = TB - w
                    mcols = slice(i0 + ioff, i0 + TB)
                    # head1 always in bank 1 of the st tile (same-bank
                    # packing of the two concurrent PE writes is fatal)
                    hoff = TB
                    st = psum.tile([128, 2 * TB], F32, tag="st",
                                   name=f"st_{b}_{ib}_{jt}")
                    nc.tensor.matmul(
                        st[:, 0:w],
                        KT[0:D, jg * 128:(jg + 1) * 128],
                        QT[0:D, mcols],
                        start=True, stop=True,
                    )
                    nc.tensor.matmul(
                        st[:, hoff:hoff + w],
                        KT[D:2 * D, jg * 128:(jg + 1) * 128],
                        QT[D:2 * D, mcols],
                        start=True, stop=True,
                    )
                    pt = work.tile([128, 2 * TB], BF16, tag="pt", bufs=6,
                                   name=f"pt_{b}_{ib}_{jt}")
                    nc.scalar.activation(pt[:, 0:hoff + w], st[:, 0:hoff + w],
                                         AF.Exp, scale=0.125)
                    if q >= 0:
                        nc.vector.tensor_mul(
                            pt[:, 0:128], pt[:, 0:128], tri_sb)
                        nc.vector.tensor_mul(
                            pt[:, hoff:hoff + 128], pt[:, hoff:hoff + 128],
                            tri_sb)
                    off = jg * VW
                    nc.tensor.matmul(
                        ot0[:, ioff:TB],
                        V[:, off:off + 128],
                        pt[:, 0:w],
                        start=(jt == 0), stop=(jt == njt - 1),
                    )
                    nc.tensor.matmul(
                        ot1[:, ioff:TB],
                        V[:, off + 128:off + 256],
                        pt[:, hoff:hoff + w],
                        start=(jt == 0), stop=(jt == njt - 1),
                    )
                # normalization: s_h at partition 64 of each ot. Drain the
                # O halves and s rows to SBUF immediately (ACT+DVE in
                # parallel) so the two ot PSUM banks release ~2.7us earlier
                # for the next block's O accumulation; the reciprocal /
                # broadcast / scale chain then runs entirely from SBUF.
                ss = work.tile([1, 2 * TB], F32, tag="ss", name=f"ss_{b}_{ib}")
                nc.scalar.copy(ss[0:1, 0:TB], ot0[64:65, :])
                nc.vector.tensor_copy(ss[0:1, TB:2 * TB], ot1[64:65, :])
                yU0 = work.tile([64, TB], F32, tag="yU0", name=f"yU0_{b}_{ib}")
                yU1 = work.tile([64, TB], F32, tag="yU1", name=f"yU1_{b}_{ib}")
                nc.vector.tensor_copy(yU0[0:64, :], ot0[0:64, :])
                nc.vector.tensor_copy(yU1[0:64, :], ot1[0:64, :])
                rr = work.tile([1, 2 * TB], F32, tag="rr", name=f"rr_{b}_{ib}")
                nc.vector.reciprocal_approx_fast(rr[0:1, 0:TB], ss[0:1, 0:TB])
                nc.vector.reciprocal_approx_fast(rr[0:1, TB:2 * TB],
                                                 ss[0:1, TB:2 * TB])
                bc = work.tile([64, 2 * TB], F32, tag="bcsb", name=f"bc_{b}_{ib}")
                nc.gpsimd.partition_broadcast(bc[0:64, 0:TB], rr[0:1, 0:TB])
                nc.gpsimd.partition_broadcast(bc[0:64, TB:2 * TB],
                                              rr[0:1, TB:2 * TB])
                nc.vector.tensor_tensor(
                    ylocT[0:64, icols], yU0[0:64, :], bc[0:64, 0:TB], ALU.mult)
                nc.vector.tensor_tensor(
                    ylocT[64:128, icols], yU1[0:64, :], bc[0:64, TB:2 * TB],
                    ALU.mult)

            # output projection for one i-block's 512 tokens; emitted AFTER
            # the next t-block's qkv so the static PE order has fill work
            # while the normalization chain runs
            def emit_outproj(b, ib, cps=(0, 1)):
                blk = b * IB + ib
                i0 = b * T + ib * TB
                icols = slice(i0, i0 + TB)
                for cp in cps:
                    yo = work.tile([128, 4 * TB], BF16, tag="yo", bufs=2,
                                   name=f"yo_{b}_{ib}_{cp}")
                    for h in range(4):
                        co = 4 * cp + h
                        yp = psum.tile([128, TB], F32, tag="mm",
                                       name=f"yp_{b}_{ib}_{co}")
                        nc.tensor.matmul(
                            yp[:, :],
                            wo_sb[:, co * 128:(co + 1) * 128],
                            ylocT[:, icols],
                            start=True, stop=True,
                        )
                        # alternate cast engine: ACT idles at block
                        # boundaries, so this doubles yp release rate
                        if h % 2 == 0:
                            nc.scalar.copy(yo[:, h * TB:(h + 1) * TB], yp[:, :])
                        else:
                            nc.vector.tensor_copy(yo[:, h * TB:(h + 1) * TB],
                                                  yp[:, :])
                    r0 = (blk * 8 + 4 * cp) * 128
                    yv = yT_d[r0:r0 + 512, :].rearrange("(c p) t -> p c t", p=128)
                    yo_v = yo[:, :].rearrange("p (c t) -> p c t", c=4)
                    nc.sync.dma_start(yv[:, :, :], yo_v[:, :, :])

            for tb in range(NTB):
                pf = [1] if tb == 0 else []
                if tb + 2 < NTB:
                    pf.append(tb + 2)  # prefetch two blocks ahead
                emit_qkv(tb, prefetch=pf)
                if tb > 0:
                    emit_outproj((tb - 1) // IB, (tb - 1) % IB)
                emit_attention(tb // IB, tb % IB)
            emit_outproj(1, IB - 1, cps=(0,))
            emit_outproj(1, IB - 1, cps=(1,))
    nc.compile()
    return nc


def _host_inputs(x, Wq, bq, Wk, bk, Wv, bv, Wo):
    """Build the 8 per-core input maps (host-side layout prep + sharding)."""
    import ml_dtypes
    bf16 = ml_dtypes.bfloat16
    xT = np.ascontiguousarray(x.reshape(BT, C).T.astype(bf16))  # [C, BT]
    # pre-tiled: xTt[tb, p, ct*TB + t] = xT[ct*128 + p, tb*TB + t]
    xTt = np.ascontiguousarray(
        xT.reshape(NKT, 128, NTB, TB).transpose(2, 1, 0, 3)
        .reshape(NTB * 128, NKT * TB))
    jj = np.arange(128, dtype=np.int32)[:, None]
    ii = np.arange(128, dtype=np.int32)[None, :]
    tri = (ii >= jj).astype(np.float32).astype(bf16)
    ident = np.eye(128, dtype=bf16)

    def wtile(W, rows):
        # device layout: w_sb[p, k*128 + j] = W[rows][j, k*128 + p]
        wT = W[rows, :].T.astype(bf16)                # [C, CL]
        return np.ascontiguousarray(
            wT.reshape(NKT, 128, CL).transpose(1, 0, 2).reshape(128, NKT * CL))

    in_maps = []
    for core in range(NCORES):
        rows = slice(core * CL, (core + 1) * CL)
        wqkv = np.concatenate(
            [wtile(Wq, rows), wtile(Wk, rows), wtile(Wv, rows)], axis=1)
        bqkv = np.stack(
            [bq[rows], bk[rows], bv[rows]], axis=1).astype(np.float32)
        in_maps.append({
            "xTt": xTt,
            "wqkv": np.ascontiguousarray(wqkv),
            "woT": np.ascontiguousarray(Wo[:, rows].T.astype(bf16)),
            "bqkv": np.ascontiguousarray(bqkv),
            "identri": np.ascontiguousarray(
                np.concatenate([ident, tri], axis=1)),
        })
    return in_maps


_NC_CACHE = None


def _get_nc():
    global _NC_CACHE
    if _NC_CACHE is None:
        _NC_CACHE = build_nc()
    return _NC_CACHE


def _run(inputs, trace=False):
    x = np.asarray(inputs["x"], np.float32)
    in_maps = _host_inputs(
        x,
        np.asarray(inputs["Wq"], np.float32), np.asarray(inputs["bq"], np.float32),
        np.asarray(inputs["Wk"], np.float32), np.asarray(inputs["bk"], np.float32),
        np.asarray(inputs["Wv"], np.float32), np.asarray(inputs["bv"], np.float32),
        np.asarray(inputs["Wo"], np.float32),
    )
    res = run_bass_kernel_spmd(_get_nc(), in_maps, list(range(NCORES)), trace=trace)
    acc = np.zeros((8, 128, NTB, TB), np.float64)
    for core in range(NCORES):
        part = res.results[core]["yTt"].astype(np.float64)
        acc += part.reshape(NTB, 8, 128, TB).transpose(1, 2, 0, 3)
    yT = acc.reshape(C, BT)
    y = yT.T.astype(np.float32) + np.asarray(inputs["bo"], np.float32)
    return y.reshape(B, T, C), res


def kernel(**inputs) -> np.ndarray:
    out, _ = _run(inputs, trace=False)
    return out


def _install_profile_hook():
    """Register the axon NTFF profile hook (the agent image ships the ctypes
    shim in trn_agent_boot but lacks the antenv.axon_hooks module)."""
    import types

    if "antenv.axon_hooks" in sys.modules:
        return
    sys.path.insert(0, "/root/.axon_site")
    from trn_agent_boot.trn_boot import _ntff_profile_via_ctypes

    mod = types.ModuleType("antenv.axon_hooks")
    hook = _ntff_profile_via_ctypes("/opt/axon/libaxon_pjrt.so")
    mod.get_axon_ntff_profile_hook = lambda: hook
    mod.set_axon_ntff_profile_hook = lambda h: None
    sys.modules["antenv.axon_hooks"] = mod
    import antenv

    antenv.axon_hooks = mod
    from concourse import bass_utils as _bu

    _bu.upload_artifacts = lambda tmpdir: tmpdir  # keep artifacts local


def kernel_profiled(**inputs):
    """Returns (output, exec_time_ns) using the NTFF profile of core 0."""
    _install_profile_hook()
    out, res = _run(inputs, trace=True)
    return out, res.exec_time_ns


# revision 28
# speedup vs baseline: 1.0968x; 1.0968x over previous
"""Causal self-attention (B=2, T=2048, C=1024, H=16) on 8 TRN2 NeuronCores.

Megatron-style tensor parallelism over heads: each core computes 2 of the 16
heads (Wq/Wk/Wv column-sharded, Wo row-sharded) and produces a partial output
projection; the partials are summed on the host (the all-reduce).

Per-core device dataflow (everything kept transposed so the PE contraction dim
is always the partition dim, no on-device transposes of x needed):
  xTt [tb*128+p, ct*512+t] host-pretiled x so each DMA descriptor moves a
      contiguous 8KB per partition; prefetched two t-blocks ahead
  QT/KT/VT = Wqkv_locT.T @ xT  (bf16 matmuls, K-tiled over C, one merged
      weight tile; PSUM drained by DVE tensor_scalar_add which also casts)
  V tiles  = PE-transpose of VT; per-head weight blocks padded to 128 cols:
      [V_h(64) | ones | junk(63)] so FWL stays enabled and the ones column
      accumulates the softmax row-sums
  S^T      = K_loc @ Q_loc^T per (batch, 128-j-tile, 512-i-block); the two
      heads run as concurrent PE row groups (contraction D=64 at rows 0-63 /
      64-127) into separate PSUM banks; on diagonal j-tiles the moving
      i-range is trimmed to the causal suffix
  P^T      = exp(S^T / 8) on ACT, one instruction per j-tile (no max
      subtraction needed: |S| < ~4); causal tri-mask multiply only on the
      128-wide diagonal chunk of each head
  O^T|s    = [V-block].T @ P^T accumulated over j in PSUM; O_h at partitions
      0-63, s_h at partition 64
  ylocT    = O^T * (1/s): s rows copied to partition 0 (ACT+DVE in
      parallel), DVE reciprocal, gpsimd partition-broadcast per head, then
      two DVE mults (head1 writes partitions 64-127 via output base shift)
  yT_part  = Wo_locT.T @ ylocT -> bf16 casts alternate ACT/DVE, batched
      DMA per 4 co-tiles
Host: y = (sum_cores yT_part).T + bo, reshape to [B, T, C].

Scheduling: emission interleaves qkv(tb) -> outproj(block tb-1) ->
attention(block tb); the attention chain is wrapped in tc.high_priority()
so the ACT exp stream stays dense while qkv/outproj matmuls fill PE gaps,
and the block-boundary normalization chain overlaps the next block's
S/exp pipeline and the previous block's output projection.
"""
import sys

if "/opt/trn_rl_repo" not in sys.path:
    sys.path.insert(0, "/opt/trn_rl_repo")

import numpy as np

import concourse.bass as bass
import concourse.tile as tile
from concourse import bacc
from concourse import mybir
from concourse.bass_utils import run_bass_kernel_spmd

F32 = mybir.dt.float32
BF16 = mybir.dt.bfloat16
AF = mybir.ActivationFunctionType
ALU = mybir.AluOpType

B, T, C, H = 2, 2048, 1024, 16
D = C // H          # 64
NCORES = 8
HL = H // NCORES    # 2 local heads
CL = C // NCORES    # 128 local channels
BT = B * T          # 4096
TB = 512            # t-block (matmul moving width, fp32 psum max)
NTB = BT // TB      # 8
NKT = C // 128      # 8 contraction tiles for projections
IB = T // TB        # 4 i-blocks per batch
NJT = T // 128      # 16 j-tiles per batch
VW = 256            # V cols per 128-j group: 2 heads x 128 padded weight cols


def build_nc() -> bass.Bass:
    nc = bacc.Bacc()

    xt_d = nc.declare_dram_parameter("xTt", [NTB * 128, NKT * TB], BF16,
                                     isOutput=False)
    wqkv_d = nc.declare_dram_parameter("wqkv", [128, 3 * C], BF16,
                                       isOutput=False)
    woT_d = nc.declare_dram_parameter("woT", [CL, C], BF16, isOutput=False)
    bqkv_d = nc.declare_dram_parameter("bqkv", [128, 3], F32, isOutput=False)
    it_d = nc.declare_dram_parameter("identri", [128, 256], BF16,
                                     isOutput=False)
    yT_d = nc.declare_dram_parameter("yTt", [NTB * 8 * 128, TB], BF16,
                                     isOutput=True)

    with tile.TileContext(nc) as tc:
        with (
            tc.tile_pool(name="const", bufs=1) as const,
            tc.tile_pool(name="work", bufs=2) as work,
            tc.tile_pool(name="psum", bufs=2, space="PSUM") as psum,
        ):
            # ---------------- constants / persistent state ----------------
            wqkv_sb = const.tile([128, 3 * C], BF16)
            bqkv_sb = const.tile([128, 3], F32)

            # first x block, split so the very first matmul starts early
            xts = []
            for tb in range(NTB):
                xts.append(work.tile([128, NKT * TB], BF16, tag="xt", bufs=3,
                                     name=f"xt_{tb}"))

            def emit_x_dma(tb):
                rows = slice(tb * 128, (tb + 1) * 128)
                if tb == 0:
                    nc.sync.dma_start(xts[0][:, 0:TB], xt_d[rows, 0:TB])
                    nc.sync.dma_start(xts[0][:, TB:4 * TB], xt_d[rows, TB:4 * TB])
                    nc.sync.dma_start(xts[0][:, 4 * TB:], xt_d[rows, 4 * TB:])
                else:
                    nc.sync.dma_start(xts[tb][:, :], xt_d[rows, :])

            emit_x_dma(0)
            nc.sync.dma_start(wqkv_sb[:, 0:128], wqkv_d[:, 0:128])
            nc.sync.dma_start(wqkv_sb[:, 128:C], wqkv_d[:, 128:C])
            nc.sync.dma_start(bqkv_sb[:, :], bqkv_d[:, :])
            nc.sync.dma_start(wqkv_sb[:, C:3 * C], wqkv_d[:, C:3 * C])
            it_sb = const.tile([128, 256], BF16)
            nc.sync.dma_start(it_sb[:, :], it_d[:, :])
            id_sb = it_sb[:, 0:128]
            tri_sb = it_sb[:, 128:256]
            wo_sb = const.tile([128, C], BF16)
            nc.sync.dma_start(wo_sb[:, :], woT_d[:, :])

            QT = const.tile([128, BT], BF16)
            KT = const.tile([128, BT], BF16)
            ylocT = const.tile([128, BT], BF16)
            V = const.tile([128, (BT // 128) * VW], BF16)
            # zero-fill once (junk weight cols stay inert), then set the
            # per-head ones columns (bf16 1.0 = 0x3F80)
            nc.gpsimd.memset(V[:, :].bitcast(mybir.dt.uint16), 0)
            for _jg in range(BT // 128):
                for _c in (_jg * VW + 64, _jg * VW + 192):
                    nc.gpsimd.memset(V[:, _c:_c + 1].bitcast(mybir.dt.uint16),
                                     0x3F80)

            # ---------------- phase 1: Q/K/V projections -------------------
            def emit_qkv(tb, prefetch=()):
                tcols = slice(tb * TB, (tb + 1) * TB)
                xt = xts[tb]
                for ptb in prefetch:
                    emit_x_dma(ptb)
                for wi, which in enumerate(("q", "k", "v")):
                    b_sb = bqkv_sb[:, wi:wi + 1]
                    ps = psum.tile([128, TB], F32, tag="mm", name=f"ps_{which}_{tb}")
                    for ct in range(NKT):
                        nc.tensor.matmul(
                            ps[:, :],
                            wqkv_sb[:, wi * C + ct * 128:wi * C + (ct + 1) * 128],
                            xt[:, ct * TB:(ct + 1) * TB],
                            start=(ct == 0), stop=(ct == NKT - 1),
                        )
                    if which == "q":
                        nc.vector.tensor_scalar_add(QT[:, tcols], ps[:, :],
                                                    b_sb[:, :])
                    elif which == "k":
                        nc.vector.tensor_scalar_add(KT[:, tcols], ps[:, :],
                                                    b_sb[:, :])
                    else:
                        vt_sb = work.tile([128, TB], BF16, tag="vtsb",
                                          name=f"vt_{tb}")
                        nc.vector.tensor_scalar_add(vt_sb[:, :], ps[:, :],
                                                    b_sb[:, :])
                        for q in range(4):
                            jg = tb * 4 + q
                            tp = psum.tile([128, 128], BF16, tag="mm",
                                           name=f"tp_{jg}")
                            off = jg * VW
                            nc.tensor.transpose(
                                tp[:, :],
                                vt_sb[:, q * 128:(q + 1) * 128],
                                id_sb,
                            )
                            # per-head blocks: [V(64) | ones | junk(63)]
                            nc.vector.tensor_copy(V[:, off:off + 64],
                                                  tp[:, 0:64])
                            nc.vector.tensor_copy(V[:, off + 128:off + 192],
                                                  tp[:, 64:128])

            # ------- phase 2+3: attention + output projection per i-block ----
            def emit_attention(b, ib):
                # The jt loop is ACT(exp)-paced; keep its whole chain ahead
                # of qkv/outproj fill work in the scheduler's priority heap.
                with tc.high_priority():
                    _emit_attention(b, ib)

            def _emit_attention(b, ib):
                blk = b * IB + ib
                i0 = b * T + ib * TB
                icols = slice(i0, i0 + TB)
                njt = 4 * (ib + 1)
                ot0 = psum.tile([128, TB], F32, tag="ot", bufs=2,
                                name=f"ot0_{b}_{ib}")
                ot1 = psum.tile([128, TB], F32, tag="ot", bufs=2,
                                name=f"ot1_{b}_{ib}")
                for jt in range(njt):
                    jg = b * NJT + jt
                    q = jt - (njt - 4)      # >= 0 -> diagonal 512-block
                    w = TB if q < 0 else TB - 128 * q
                    ioff = TB - w
                    mcols = slice(i0 + ioff, i0 + TB)
                    # head1 always in bank 1 of the st tile (same-bank
                    # packing of the two concurrent PE writes is fatal)
                    hoff = TB
                    st = psum.tile([128, 2 * TB], F32, tag="st",
                                   name=f"st_{b}_{ib}_{jt}")
                    nc.tensor.matmul(
                        st[:, 0:w],
                        KT[0:D, jg * 128:(jg + 1) * 128],
                        QT[0:D, mcols],
                        start=True, stop=True,
                    )
                    nc.tensor.matmul(
                        st[:, hoff:hoff + w],
                        KT[D:2 * D, jg * 128:(jg + 1) * 128],
                        QT[D:2 * D, mcols],
                        start=True, stop=True,
                    )
                    pt = work.tile([128, 2 * TB], BF16, tag="pt", bufs=6,
                                   name=f"pt_{b}_{ib}_{jt}")
                    nc.scalar.activation(pt[:, 0:hoff + w], st[:, 0:hoff + w],
                                         AF.Exp, scale=0.125)
                    if q >= 0:
                        nc.vector.tensor_mul(
                            pt[:, 0:128], pt[:, 0:128], tri_sb)
                        nc.vector.tensor_mul(
                            pt[:, hoff:hoff + 128], pt[:, hoff:hoff + 128],
                            tri_sb)
                    off = jg * VW
                    nc.tensor.matmul(
                        ot0[:, ioff:TB],
                        V[:, off:off + 128],
                        pt[:, 0:w],
                        start=(jt == 0), stop=(jt == njt - 1),
                    )
                    nc.tensor.matmul(
                        ot1[:, ioff:TB],
                        V[:, off + 128:off + 256],
                        pt[:, hoff:hoff + w],
                        start=(jt == 0), stop=(jt == njt - 1),
                    )
                # normalization: s_h at partition 64 of each ot; move both
                # to partition 0 (DVE copy handles the 32-aligned shift),
                # one reciprocal per head, broadcast, scale each O half.
                ss = work.tile([1, 2 * TB], F32, tag="ss", name=f"ss_{b}_{ib}")
                nc.scalar.copy(ss[0:1, 0:TB], ot0[64:65, :])
                nc.vector.tensor_copy(ss[0:1, TB:2 * TB], ot1[64:65, :])
                rr = work.tile([1, 2 * TB], F32, tag="rr", name=f"rr_{b}_{ib}")
                nc.vector.reciprocal_approx_fast(rr[0:1, 0:TB], ss[0:1, 0:TB])
                nc.vector.reciprocal_approx_fast(rr[0:1, TB:2 * TB],
                                                 ss[0:1, TB:2 * TB])
                bc = work.tile([64, 2 * TB], F32, tag="bcsb", name=f"bc_{b}_{ib}")
                nc.gpsimd.partition_broadcast(bc[0:64, 0:TB], rr[0:1, 0:TB])
                nc.gpsimd.partition_broadcast(bc[0:64, TB:2 * TB],
                                              rr[0:1, TB:2 * TB])
                nc.vector.tensor_tensor(
                    ylocT[0:64, icols], ot0[0:64, :], bc[0:64, 0:TB], ALU.mult)
                nc.vector.tensor_tensor(
                    ylocT[64:128, icols], ot1[0:64, :], bc[0:64, TB:2 * TB],
                    ALU.mult)

            # output projection for one i-block's 512 tokens; emitted AFTER
            # the next t-block's qkv so the static PE order has fill work
            # while the normalization chain runs
            def emit_outproj(b, ib, cps=(0, 1)):
                blk = b * IB + ib
                i0 = b * T + ib * TB
                icols = slice(i0, i0 + TB)
                for cp in cps:
                    yo = work.tile([128, 4 * TB], BF16, tag="yo", bufs=2,
                                   name=f"yo_{b}_{ib}_{cp}")
                    for h in range(4):
                        co = 4 * cp + h
                        yp = psum.tile([128, TB], F32, tag="mm",
                                       name=f"yp_{b}_{ib}_{co}")
                        nc.tensor.matmul(
                            yp[:, :],
                            wo_sb[:, co * 128:(co + 1) * 128],
                            ylocT[:, icols],
                            start=True, stop=True,
                        )
                        # alternate cast engine: ACT idles at block
                        # boundaries, so this doubles yp release rate
                        if h % 2 == 0:
                            nc.scalar.copy(yo[:, h * TB:(h + 1) * TB], yp[:, :])
                        else:
                            nc.vector.tensor_copy(yo[:, h * TB:(h + 1) * TB],
                                                  yp[:, :])
                    r0 = (blk * 8 + 4 * cp) * 128
                    yv = yT_d[r0:r0 + 512, :].rearrange("(c p) t -> p c t", p=128)
                    yo_v = yo[:, :].rearrange("p (c t) -> p c t", c=4)
                    nc.sync.dma_start(yv[:, :, :], yo_v[:, :, :])

            for tb in range(NTB):
                pf = [1] if tb == 0 else []
                if tb + 2 < NTB:
                    pf.append(tb + 2)  # prefetch two blocks ahead
                emit_qkv(tb, prefetch=pf)
                if tb > 0:
                    emit_outproj((tb - 1) // IB, (tb - 1) % IB)
                emit_attention(tb // IB, tb % IB)
            emit_outproj(1, IB - 1, cps=(0,))
            emit_outproj(1, IB - 1, cps=(1,))
    nc.compile()
    return nc


def _host_inputs(x, Wq, bq, Wk, bk, Wv, bv, Wo):
    """Build the 8 per-core input maps (host-side layout prep + sharding)."""
    import ml_dtypes
    bf16 = ml_dtypes.bfloat16
    xT = np.ascontiguousarray(x.reshape(BT, C).T.astype(bf16))  # [C, BT]
    # pre-tiled: xTt[tb, p, ct*TB + t] = xT[ct*128 + p, tb*TB + t]
    xTt = np.ascontiguousarray(
        xT.reshape(NKT, 128, NTB, TB).transpose(2, 1, 0, 3)
        .reshape(NTB * 128, NKT * TB))
    jj = np.arange(128, dtype=np.int32)[:, None]
    ii = np.arange(128, dtype=np.int32)[None, :]
    tri = (ii >= jj).astype(np.float32).astype(bf16)
    ident = np.eye(128, dtype=bf16)

    def wtile(W, rows):
        # device layout: w_sb[p, k*128 + j] = W[rows][j, k*128 + p]
        wT = W[rows, :].T.astype(bf16)                # [C, CL]
        return np.ascontiguousarray(
            wT.reshape(NKT, 128, CL).transpose(1, 0, 2).reshape(128, NKT * CL))

    in_maps = []
    for core in range(NCORES):
        rows = slice(core * CL, (core + 1) * CL)
        wqkv = np.concatenate(
            [wtile(Wq, rows), wtile(Wk, rows), wtile(Wv, rows)], axis=1)
        bqkv = np.stack(
            [bq[rows], bk[rows], bv[rows]], axis=1).astype(np.float32)
        in_maps.append({
            "xTt": xTt,
            "wqkv": np.ascontiguousarray(wqkv),
            "woT": np.ascontiguousarray(Wo[:, rows].T.astype(bf16)),
            "bqkv": np.ascontiguousarray(bqkv),
            "identri": np.ascontiguousarray(
                np.concatenate([ident, tri], axis=1)),
        })
    return in_maps


_NC_CACHE = None


def _get_nc():
    global _NC_CACHE
    if _NC_CACHE is None:
        _NC_CACHE = build_nc()
    return _NC_CACHE


def _run(inputs, trace=False):
    x = np.asarray(inputs["x"], np.float32)
    in_maps = _host_inputs(
        x,
        np.asarray(inputs["Wq"], np.float32), np.asarray(inputs["bq"], np.float32),
        np.asarray(inputs["Wk"], np.float32), np.asarray(inputs["bk"], np.float32),
        np.asarray(inputs["Wv"], np.float32), np.asarray(inputs["bv"], np.float32),
        np.asarray(inputs["Wo"], np.float32),
    )
    res = run_bass_kernel_spmd(_get_nc(), in_maps, list(range(NCORES)), trace=trace)
    acc = np.zeros((8, 128, NTB, TB), np.float64)
    for core in range(NCORES):
        part = res.results[core]["yTt"].astype(np.float64)
        acc += part.reshape(NTB, 8, 128, TB).transpose(1, 2, 0, 3)
    yT = acc.reshape(C, BT)
    y = yT.T.astype(np.float32) + np.asarray(inputs["bo"], np.float32)
    return y.reshape(B, T, C), res


def kernel(**inputs) -> np.ndarray:
    out, _ = _run(inputs, trace=False)
    return out


def _install_profile_hook():
    """Register the axon NTFF profile hook (the agent image ships the ctypes
    shim in trn_agent_boot but lacks the antenv.axon_hooks module)."""
    import types

    if "antenv.axon_hooks" in sys.modules:
        return
    sys.path.insert(0, "/root/.axon_site")
    from trn_agent_boot.trn_boot import _ntff_profile_via_ctypes

    mod = types.ModuleType("antenv.axon_hooks")
    hook = _ntff_profile_via_ctypes("/opt/axon/libaxon_pjrt.so")
    mod.get_axon_ntff_profile_hook = lambda: hook
    mod.set_axon_ntff_profile_hook = lambda h: None
    sys.modules["antenv.axon_hooks"] = mod
    import antenv

    antenv.axon_hooks = mod
    from concourse import bass_utils as _bu

    _bu.upload_artifacts = lambda tmpdir: tmpdir  # keep artifacts local


def kernel_profiled(**inputs):
    """Returns (output, exec_time_ns) using the NTFF profile of core 0."""
    _install_profile_hook()
    out, res = _run(inputs, trace=True)
    return out, res.exec_time_ns
